# revision 11
# baseline (speedup 1.0000x reference)
"""Trainium2 Bass kernel for nn_BaseIterativeNet (pose feedback loss).

reference semantics:
  features = broadcast(pred_poses.reshape(L, 3K, 1, 1), (L, 3K, D, D))
  loss     = sum(ious * ((pf - mean_k exp(-d2/(2a+eps)))**2 + bce))

Sharding: data-parallel over L=512 -> 8 cores x 64 rows. Targets/areas
replicated. Per-core partial loss (per-row sums) finished on host.
"""
import os
import sys
import numpy as np

for _p in ("/root/.axon_site", "/root/.axon_site/_ro/trn_rl_repo",
           "/root/.axon_site/_ro/pypackages", "/opt/trn_rl_repo"):
    if _p not in sys.path:
        sys.path.append(_p)

L, LP, K = 512, 64, 17
D = 56
C = D * D            # 3136
CH = 3 * K           # 51
M = 8                # cores
LS = L // M          # 64 rows per core
ROWS = LS * CH       # 3264 feature rows per core
NT = (ROWS + 127) // 128   # 26 value tiles (25 full + 1 of 64 rows)
EPS_AREA = 1e-6
EPS_BCE = 1e-7

# tunables (overridable for experiments via KCFG env: "key=val,key=val")
CFG = {
    "rows_per_tile": 1,   # feature rows per SBUF partition per tile (1 or 2)
    "chunk": 0,           # if >0: generate only `chunk` cols, DMA replicates
    "alt_rings": 1,       # alternate feat DMAs between sync/scalar HWDGE rings
    "bufs": 8,            # big-tile pool depth
    "loss_on_gpsimd": 1,  # small loss DMAs on SWDGE to keep HWDGE rings clean
    "host_bcast": 1,      # replicate targets/areas on host (vs 0-step DMA src)
    "gen_gpsimd": 1,      # generation on DVE+GpSimd (vs DVE+ACT)
}
for kv in os.environ.get("KCFG", "").split(","):
    if "=" in kv:
        k, v = kv.split("=")
        CFG[k.strip()] = int(v)

_NC = None


def _build():
    from concourse import bacc, bass, mybir, tile

    f32 = mybir.dt.float32
    nc = bacc.Bacc("TRN2", target_bir_lowering=False, debug=False, num_devices=M)

    rpt = CFG["rows_per_tile"]
    chunk = CFG["chunk"]
    assert ROWS % (128 * rpt) != 0 or True
    n_full = ROWS // (128 * rpt)          # full tiles
    rem = ROWS - n_full * 128 * rpt       # leftover rows

    ppv = nc.dram_tensor("ppv", [128, NT], f32, kind="ExternalInput")
    pp = nc.dram_tensor("pp", [LS, CH], f32, kind="ExternalInput")
    hb = CFG["host_bcast"]
    tp = nc.dram_tensor("tp", [LS, LP * CH] if hb else [LP, CH], f32,
                        kind="ExternalInput")
    pf = nc.dram_tensor("pf", [LS, 1], f32, kind="ExternalInput")
    iou = nc.dram_tensor("iou", [LS, LP], f32, kind="ExternalInput")
    ta = nc.dram_tensor("ta", [LS if hb else 1, LP], f32, kind="ExternalInput")
    feat = nc.dram_tensor("feat", [ROWS, C], f32, kind="ExternalOutput")
    pl = nc.dram_tensor("pl", [LS, 1], f32, kind="ExternalOutput")

    Exp = mybir.ActivationFunctionType.Exp
    Ln = mybir.ActivationFunctionType.Ln
    Alu = mybir.AluOpType
    X = mybir.AxisListType.X

    with tile.TileContext(nc) as tc:
        with tc.tile_pool(name="small", bufs=1) as sp, \
             tc.tile_pool(name="big", bufs=CFG["bufs"]) as bp:
            ldma = nc.gpsimd if CFG["loss_on_gpsimd"] else nc.sync
            # ---- value-vector load first: unblocks feature tiles ----
            vsb = sp.tile([128, NT], f32)
            nc.sync.dma_start(vsb[:], ppv.ap())

            def loss_loads():
                pp_sb = sp.tile([LS, CH], f32)
                ldma.dma_start(pp_sb[:], pp.ap())
                pf_sb = sp.tile([LS, 1], f32)
                ldma.dma_start(pf_sb[:], pf.ap())
                iou_sb = sp.tile([LS, LP], f32)
                ldma.dma_start(iou_sb[:], iou.ap())
                tpb = sp.tile([LS, LP * CH], f32)
                tab = sp.tile([LS, LP], f32)
                if CFG["host_bcast"]:
                    ldma.dma_start(tpb[:], tp.ap())
                    ldma.dma_start(tab[:], ta.ap())
                else:
                    # broadcast to all partitions via 0-step DMA src
                    ldma.dma_start(tpb[:], bass.AP(tp, 0, [[0, LS], [1, LP * CH]]))
                    ldma.dma_start(tab[:], bass.AP(ta, 0, [[0, LS], [1, LP]]))
                return pp_sb, pf_sb, iou_sb, tpb, tab

            pp_sb, pf_sb, iou_sb, tpb, tab = loss_loads()

            # ---- features: per-partition broadcast + DMA out ----
            gen2 = nc.gpsimd if CFG["gen_gpsimd"] else nc.scalar

            def gen_and_store(idx, row0, rows, width):
                """one tile: rows from row0, value col = idx in vsb"""
                big = bp.tile([128, rpt * (chunk or C)], f32, tag="big")
                segs = []
                for j in range(rpt):
                    if row0 + j * 128 >= ROWS:
                        break
                    r = min(rows, 128)
                    segs.append((j, r))
                w = chunk or C
                for j, r in segs:
                    src = vsb[0:r, idx + j:idx + j + 1].broadcast_to((r, w))
                    dst = big[0:r, j * w:(j + 1) * w]
                    if idx % 2 == 0:
                        nc.vector.tensor_copy(dst, src)
                    elif CFG["gen_gpsimd"]:
                        gen2.tensor_copy(dst, src)
                    else:
                        nc.scalar.copy(dst, src)
                # DMA out (possibly replicating chunk -> C via 0-step src)
                eng = nc.scalar if (CFG["alt_rings"] and (idx // rpt) % 2) else nc.sync
                if rpt > 1:
                    # one fused 3-dim DMA: dst (p, j, col), src (p, j, col)
                    assert not chunk and all(r == 128 for _, r in segs)
                    nj = len(segs)
                    d_ap = bass.AP(feat, row0 * C,
                                   [[C, 128], [128 * C, nj], [1, C]])
                    s_ap = big[:].rearrange("p (j c) -> p j c", c=C)[:, 0:nj, :]
                    eng.dma_start(d_ap, s_ap)
                else:
                    r = segs[0][1]
                    doff = row0 * C
                    if chunk:
                        rep = C // chunk
                        d_ap = bass.AP(feat, doff, [[C, r], [chunk, rep], [1, chunk]])
                        s_ap = big[0:r, 0:w].unsqueeze(1) \
                            .broadcast_to((r, rep, chunk))
                        eng.dma_start(d_ap, s_ap)
                    else:
                        eng.dma_start(bass.AP(feat, doff, [[C, r], [1, C]]),
                                      big[0:r, 0:w])

            for t in range(n_full):
                gen_and_store(t * rpt, t * rpt * 128, 128, chunk or C)
            # tail tiles (single-row-per-partition)
            done = n_full * rpt * 128
            ti = n_full * rpt
            while done < ROWS:
                rows = min(128, ROWS - done)
                big = bp.tile([128, rpt * (chunk or C)], f32, tag="big")
                w = chunk or C
                src = vsb[0:rows, ti:ti + 1].broadcast_to((rows, w))
                nc.vector.tensor_copy(big[0:rows, 0:w], src)
                eng = nc.scalar if (CFG["alt_rings"] and ti % 2) else nc.sync
                if chunk:
                    rep = C // chunk
                    eng.dma_start(
                        bass.AP(feat, done * C, [[C, rows], [chunk, rep], [1, chunk]]),
                        big[0:rows, 0:w].unsqueeze(1).broadcast_to((rows, rep, chunk)))
                else:
                    eng.dma_start(bass.AP(feat, done * C, [[C, rows], [1, C]]),
                                  big[0:rows, 0:w])
                done += rows
                ti += 1

            # ---- loss pipeline (64 partitions, tiny) ----
            invs = sp.tile([LS, LP], f32)
            nc.vector.tensor_scalar(invs[:], tab[:], 2.0, EPS_AREA, Alu.mult, Alu.add)
            nc.vector.reciprocal(invs[:], invs[:])  # 1/(2a+eps)

            pp3 = pp_sb[:].rearrange("p (k c) -> p k c", c=3)
            tp4 = tpb[:].rearrange("p (lp k c) -> p lp k c", k=K, c=3)
            px_r = pp3[:, :, 0].unsqueeze(1).broadcast_to((LS, LP, K))
            py_r = pp3[:, :, 1].unsqueeze(1).broadcast_to((LS, LP, K))
            tx = tp4[:, :, :, 0]
            ty = tp4[:, :, :, 1]
            tv = tp4[:, :, :, 2]

            dx = sp.tile([LS, LP * K], f32)
            dy = sp.tile([LS, LP * K], f32)
            dx3 = dx[:].rearrange("p (lp k) -> p lp k", k=K)
            dy3 = dy[:].rearrange("p (lp k) -> p lp k", k=K)
            nc.vector.tensor_sub(dx3, px_r, tx)
            nc.vector.tensor_sub(dy3, py_r, ty)
            nc.scalar.square(dx[:], dx[:])
            nc.scalar.square(dy[:], dy[:])
            nc.vector.tensor_add(dx[:], dx[:], dy[:])  # d2
            invs_r = invs[:].unsqueeze(2).broadcast_to((LS, LP, K))
            nc.vector.tensor_mul(dx3, dx3, invs_r)
            nc.scalar.activation(dx[:], dx[:], Exp, scale=-1.0)  # exp(-d2/(2a+eps))
            tf = sp.tile([LS, LP], f32)
            nc.vector.tensor_reduce(tf[:], dx3, X, Alu.add)
            fb = sp.tile([LS, LP], f32)
            # (sum/K - pf)  (== -(pf - mean), squared below)
            nc.vector.tensor_scalar(fb[:], tf[:], 1.0 / K, pf_sb[:, 0:1],
                                    Alu.mult, Alu.subtract)
            nc.scalar.square(fb[:], fb[:])

            # bce = -(sum_k l1p + sum_k tv*(lpv-l1p)) / K
            pvc = sp.tile([LS, K], f32)
            nc.vector.tensor_scalar(pvc[:], pp3[:, :, 2], EPS_BCE, 1.0 - EPS_BCE,
                                    Alu.max, Alu.min)
            lpv = sp.tile([LS, K], f32)
            nc.scalar.activation(lpv[:], pvc[:], Ln)
            om = sp.tile([LS, K], f32)
            nc.vector.tensor_scalar(om[:], pvc[:], -1.0, 1.0, Alu.mult, Alu.add)
            l1p = sp.tile([LS, K], f32)
            nc.scalar.activation(l1p[:], om[:], Ln)
            s1 = sp.tile([LS, 1], f32)
            nc.vector.tensor_reduce(s1[:], l1p[:], X, Alu.add)
            dl = sp.tile([LS, K], f32)
            nc.vector.tensor_sub(dl[:], lpv[:], l1p[:])
            prod = sp.tile([LS, LP * K], f32)
            prod3 = prod[:].rearrange("p (lp k) -> p lp k", k=K)
            dl_r = dl[:].unsqueeze(1).broadcast_to((LS, LP, K))
            nc.vector.tensor_mul(prod3, tv, dl_r)
            rp = sp.tile([LS, LP], f32)
            nc.vector.tensor_reduce(rp[:], prod3, X, Alu.add)
            nc.vector.tensor_scalar(rp[:], rp[:], s1[:, 0:1], -1.0 / K,
                                    Alu.add, Alu.mult)  # bce
            nc.vector.tensor_add(fb[:], fb[:], rp[:])
            nc.vector.tensor_mul(fb[:], iou_sb[:], fb[:])
            pl_sb = sp.tile([LS, 1], f32)
            nc.vector.tensor_reduce(pl_sb[:], fb[:], X, Alu.add)
            ldma.dma_start(pl.ap(), pl_sb[:])

    nc.compile()
    return nc


def _get_nc():
    global _NC
    if _NC is None:
        _NC = _build()
    return _NC


def kernel(pred_poses, target_poses, pred_feedbacks, ious, target_areas,
           features_dim=56, **_unused):
    from concourse import bass_utils

    nc = _get_nc()
    pp_all = np.ascontiguousarray(pred_poses, dtype=np.float32).reshape(L, CH)
    tp_full = np.ascontiguousarray(target_poses, dtype=np.float32).reshape(LP, CH)
    pf_all = np.ascontiguousarray(pred_feedbacks, dtype=np.float32).reshape(L, 1)
    iou_all = np.ascontiguousarray(ious, dtype=np.float32)
    ta_full = np.ascontiguousarray(target_areas, dtype=np.float32).reshape(1, LP)
    if CFG["host_bcast"]:
        tp_full = np.ascontiguousarray(
            np.broadcast_to(tp_full.reshape(1, LP * CH), (LS, LP * CH)))
        ta_full = np.ascontiguousarray(np.broadcast_to(ta_full, (LS, LP)))

    in_maps = []
    for c in range(M):
        sl = slice(c * LS, (c + 1) * LS)
        ppc = pp_all[sl]
        flat = np.zeros((NT * 128,), np.float32)
        flat[:ROWS] = ppc.reshape(-1)
        ppv = np.ascontiguousarray(flat.reshape(NT, 128).T)  # ppv[p,t]=flat[t*128+p]
        in_maps.append({
            "ppv": ppv, "pp": ppc, "tp": tp_full, "pf": pf_all[sl],
            "iou": np.ascontiguousarray(iou_all[sl]), "ta": ta_full,
        })

    res = bass_utils.run_bass_kernel_spmd(nc, in_maps, core_ids=list(range(M)))
    feat = np.concatenate(
        [r["feat"].reshape(LS, CH, D, D) for r in res.results], axis=0)
    loss = np.float32(np.sum(
        np.concatenate([r["pl"].reshape(-1) for r in res.results]),
        dtype=np.float32))
    return feat, loss


# revision 12
# speedup vs baseline: 1.2673x; 1.2673x over previous
"""Trainium2 Bass kernel for nn_BaseIterativeNet (pose feedback loss).

reference semantics:
  features = broadcast(pred_poses.reshape(L, 3K, 1, 1), (L, 3K, D, D))
  loss     = sum(ious * ((pf - mean_k exp(-d2/(2a+eps)))**2 + bce))

Sharding: data-parallel over L=512 -> 8 cores x 64 rows. Targets/areas
replicated (host-packed into one aux tensor). Per-core per-row loss sums
are finished on host.

Feature path: each output row r (of 3264 per core) is input value
pp_flat[r] broadcast over 3136 spatial elements. Values are host-laid-out
partition-major ("ppv"), broadcast on-chip with per-partition-scalar
copies, and DMA'd out on both HWDGE rings (sync + scalar).
"""
import os
import sys
import numpy as np

for _p in ("/root/.axon_site", "/root/.axon_site/_ro/trn_rl_repo",
           "/root/.axon_site/_ro/pypackages", "/opt/trn_rl_repo"):
    if _p not in sys.path:
        sys.path.append(_p)

L, LP, K = 512, 64, 17
D = 56
C = D * D            # 3136
CH = 3 * K           # 51
M = 8                # cores
LS = L // M          # 64 rows per core
ROWS = LS * CH       # 3264 feature rows per core
NT = (ROWS + 127) // 128   # 26 value columns (25 full + 1 of 64 rows)
EPS_AREA = 1e-6
EPS_BCE = 1e-7
# packed aux input layout (columns): tpb | pp | pf | iou | ta
AUX_W = LP * CH + CH + 1 + LP + LP   # 3444

# tunables (overridable for experiments via KCFG env: "key=val,key=val")
CFG = {
    "rpt": 2,        # feature rows per partition in steady-state tiles
    "fast_head": 2,  # leading 1-row tiles w/ chunk-replicated DMA (fast start)
    "fast_tail": 1,  # use chunked gen for the trailing 1-row tiles too
    "chunk": 448,    # generated columns for chunked tiles (DMA replicates x7)
    "bufs": 6,       # big-tile pool depth
    "gen": "dve",    # dve | alt (dve+act) generation
}
for kv in os.environ.get("KCFG", "").split(","):
    if "=" in kv:
        k, v = kv.split("=")
        CFG[k.strip()] = v if k.strip() == "gen" else int(v)

_NC = None


def _build():
    from concourse import bacc, bass, mybir, tile

    f32 = mybir.dt.float32
    nc = bacc.Bacc("TRN2", target_bir_lowering=False, debug=False, num_devices=M)

    ppv = nc.dram_tensor("ppv", [128, NT], f32, kind="ExternalInput")
    aux = nc.dram_tensor("aux", [LS, AUX_W], f32, kind="ExternalInput")
    feat = nc.dram_tensor("feat", [ROWS, C], f32, kind="ExternalOutput")
    pl = nc.dram_tensor("pl", [LS, 1], f32, kind="ExternalOutput")

    Exp = mybir.ActivationFunctionType.Exp
    Ln = mybir.ActivationFunctionType.Ln
    Alu = mybir.AluOpType
    X = mybir.AxisListType.X

    rpt = CFG["rpt"]
    chunk = CFG["chunk"]
    rep = C // chunk if chunk else 1
    assert chunk == 0 or C % chunk == 0

    # ---- tile plan: list of (vidx, row0, nrows, chunked) ----
    plan = []
    row = 0
    for _ in range(CFG["fast_head"]):
        if row + 128 <= ROWS:
            plan.append((row // 128, row, 128, bool(chunk)))
            row += 128
    n_steady = (ROWS - row) // (128 * rpt)
    # leave the remainder for 1-row tail tiles
    for _ in range(n_steady):
        plan.append((row // 128, row, 128 * rpt, False))
        row += 128 * rpt
    while row < ROWS:
        r = min(128, ROWS - row)
        plan.append((row // 128, row, r, bool(chunk and CFG["fast_tail"])))
        row += r

    with tile.TileContext(nc) as tc:
        with tc.tile_pool(name="small", bufs=1) as sp, \
             tc.tile_pool(name="big", bufs=CFG["bufs"]) as bp:
            # ---- input loads (sync ring, ahead of feature DMAs) ----
            vsb = sp.tile([128, NT], f32)
            nc.sync.dma_start(vsb[:], ppv.ap())
            aux_sb = sp.tile([LS, AUX_W], f32)
            nc.sync.dma_start(aux_sb[:], aux.ap())
            o_tp = 0
            o_pp = o_tp + LP * CH
            o_pf = o_pp + CH
            o_iou = o_pf + 1
            o_ta = o_iou + LP
            tpb = aux_sb[:, o_tp:o_pp]
            pp_sb = aux_sb[:, o_pp:o_pf]
            pf_sb = aux_sb[:, o_pf:o_iou]
            iou_sb = aux_sb[:, o_iou:o_ta]
            tab = aux_sb[:, o_ta:o_ta + LP]

            # ---- features ----
            ndma = 0
            for vidx, row0, nrows, chunked in plan:
                nsegs = (nrows + 127) // 128
                w = chunk if chunked else C
                big = bp.tile([128, rpt * C] if not chunked else [128, chunk],
                              f32, tag="bigc" if chunked else "big")
                for j in range(nsegs):
                    r = min(128, nrows - j * 128)
                    src = vsb[0:r, vidx + j:vidx + j + 1].broadcast_to((r, w))
                    dst = big[0:r, j * w:(j + 1) * w]
                    if CFG["gen"] == "alt" and vidx % 2 == 1:
                        nc.scalar.copy(dst, src)
                    else:
                        nc.vector.tensor_copy(dst, src)
                eng = nc.scalar if ndma % 2 else nc.sync
                ndma += 1
                if chunked:
                    assert nsegs == 1
                    r = nrows
                    d_ap = bass.AP(feat, row0 * C,
                                   [[C, r], [chunk, rep], [1, chunk]])
                    s_ap = big[0:r, 0:chunk].unsqueeze(1) \
                        .broadcast_to((r, rep, chunk))
                    eng.dma_start(d_ap, s_ap)
                elif nsegs == 1:
                    r = nrows
                    eng.dma_start(bass.AP(feat, row0 * C, [[C, r], [1, C]]),
                                  big[0:r, 0:C])
                else:
                    d_ap = bass.AP(feat, row0 * C,
                                   [[C, 128], [128 * C, nsegs], [1, C]])
                    s_ap = big[:].rearrange("p (j c) -> p j c", c=C)[:, 0:nsegs, :]
                    eng.dma_start(d_ap, s_ap)

            # ---- loss pipeline (64 partitions, tiny; overlaps DMA stream) ----
            invs = sp.tile([LS, LP], f32)
            nc.vector.tensor_scalar(invs[:], tab, 2.0, EPS_AREA, Alu.mult, Alu.add)
            nc.vector.reciprocal(invs[:], invs[:])  # 1/(2a+eps)

            pp3 = pp_sb.rearrange("p (k c) -> p k c", c=3)
            tp4 = tpb.rearrange("p (lp k c) -> p lp k c", k=K, c=3)
            px_r = pp3[:, :, 0].unsqueeze(1).broadcast_to((LS, LP, K))
            py_r = pp3[:, :, 1].unsqueeze(1).broadcast_to((LS, LP, K))
            tx = tp4[:, :, :, 0]
            ty = tp4[:, :, :, 1]
            tv = tp4[:, :, :, 2]

            dx = sp.tile([LS, LP * K], f32)
            dy = sp.tile([LS, LP * K], f32)
            dx3 = dx[:].rearrange("p (lp k) -> p lp k", k=K)
            dy3 = dy[:].rearrange("p (lp k) -> p lp k", k=K)
            nc.vector.tensor_sub(dx3, px_r, tx)
            nc.vector.tensor_sub(dy3, py_r, ty)
            nc.scalar.square(dx[:], dx[:])
            nc.scalar.square(dy[:], dy[:])
            nc.vector.tensor_add(dx[:], dx[:], dy[:])  # d2
            invs_r = invs[:].unsqueeze(2).broadcast_to((LS, LP, K))
            nc.vector.tensor_mul(dx3, dx3, invs_r)
            nc.scalar.activation(dx[:], dx[:], Exp, scale=-1.0)  # exp(-d2/(2a+eps))
            tf = sp.tile([LS, LP], f32)
            nc.vector.tensor_reduce(tf[:], dx3, X, Alu.add)
            fb = sp.tile([LS, LP], f32)
            # (sum/K - pf)  (== -(pf - mean), squared below)
            nc.vector.tensor_scalar(fb[:], tf[:], 1.0 / K, pf_sb,
                                    Alu.mult, Alu.subtract)
            nc.scalar.square(fb[:], fb[:])

            # bce = -(sum_k l1p + sum_k tv*(lpv-l1p)) / K
            pvc = sp.tile([LS, K], f32)
            nc.vector.tensor_scalar(pvc[:], pp3[:, :, 2], EPS_BCE, 1.0 - EPS_BCE,
                                    Alu.max, Alu.min)
            lpv = sp.tile([LS, K], f32)
            nc.scalar.activation(lpv[:], pvc[:], Ln)
            om = sp.tile([LS, K], f32)
            nc.vector.tensor_scalar(om[:], pvc[:], -1.0, 1.0, Alu.mult, Alu.add)
            l1p = sp.tile([LS, K], f32)
            nc.scalar.activation(l1p[:], om[:], Ln)
            s1 = sp.tile([LS, 1], f32)
            nc.vector.tensor_reduce(s1[:], l1p[:], X, Alu.add)
            dl = sp.tile([LS, K], f32)
            nc.vector.tensor_sub(dl[:], lpv[:], l1p[:])
            prod = sp.tile([LS, LP * K], f32)
            prod3 = prod[:].rearrange("p (lp k) -> p lp k", k=K)
            dl_r = dl[:].unsqueeze(1).broadcast_to((LS, LP, K))
            nc.vector.tensor_mul(prod3, tv, dl_r)
            rp = sp.tile([LS, LP], f32)
            nc.vector.tensor_reduce(rp[:], prod3, X, Alu.add)
            nc.vector.tensor_scalar(rp[:], rp[:], s1[:, 0:1], -1.0 / K,
                                    Alu.add, Alu.mult)  # bce
            nc.vector.tensor_add(fb[:], fb[:], rp[:])
            nc.vector.tensor_mul(fb[:], iou_sb, fb[:])
            pl_sb = sp.tile([LS, 1], f32)
            nc.vector.tensor_reduce(pl_sb[:], fb[:], X, Alu.add)
            nc.sync.dma_start(pl.ap(), pl_sb[:])

    nc.compile()
    return nc


def _get_nc():
    global _NC
    if _NC is None:
        _NC = _build()
    return _NC


def kernel(pred_poses, target_poses, pred_feedbacks, ious, target_areas,
           features_dim=56, **_unused):
    from concourse import bass_utils

    nc = _get_nc()
    pp_all = np.ascontiguousarray(pred_poses, dtype=np.float32).reshape(L, CH)
    tp_flat = np.ascontiguousarray(target_poses, dtype=np.float32).reshape(-1)
    pf_all = np.ascontiguousarray(pred_feedbacks, dtype=np.float32).reshape(L)
    iou_all = np.ascontiguousarray(ious, dtype=np.float32)
    ta_flat = np.ascontiguousarray(target_areas, dtype=np.float32).reshape(-1)

    in_maps = []
    for c in range(M):
        sl = slice(c * LS, (c + 1) * LS)
        ppc = pp_all[sl]
        flat = np.zeros((NT * 128,), np.float32)
        flat[:ROWS] = ppc.reshape(-1)
        ppv = np.ascontiguousarray(flat.reshape(NT, 128).T)  # ppv[p,t]=flat[t*128+p]
        auxm = np.empty((LS, AUX_W), np.float32)
        o = LP * CH
        auxm[:, :o] = tp_flat[None, :]
        auxm[:, o:o + CH] = ppc
        auxm[:, o + CH] = pf_all[sl]
        auxm[:, o + CH + 1:o + CH + 1 + LP] = iou_all[sl]
        auxm[:, o + CH + 1 + LP:] = ta_flat[None, :]
        in_maps.append({"ppv": ppv, "aux": auxm})

    res = bass_utils.run_bass_kernel_spmd(nc, in_maps, core_ids=list(range(M)))
    feat = np.concatenate(
        [r["feat"].reshape(LS, CH, D, D) for r in res.results], axis=0)
    loss = np.float32(np.sum(
        np.concatenate([r["pl"].reshape(-1) for r in res.results]),
        dtype=np.float32))
    return feat, loss
